# revision 47
# baseline (speedup 1.0000x reference)
"""Low-rank layer y = x @ (U diag(s) V^T)^T on 8 TRN2 NeuronCores.

Data-parallel over batch (1 batch/core). Two thin matmuls per core:
  stage 1: t[r, n]  = sum_i (V*s)[i, r] * x[n, i]   (contraction i on partitions)
  stage 2: y[n, o]  = sum_r t[r, n] * U[o, r]       (contraction r on partitions)

Software-pipelined over 4 token groups of 512 so stage-2 of group g overlaps
stage-1 of group g+1: the tensor engine never waits on a phase boundary, input
DMA (SP ring) overlaps output DMA (ACT ring), and PSUM is split 2 banks
(stage-1 accumulators) + 6 banks (stage-2 output tiles). y is stored bf16 and
upcast on host, halving HBM write traffic. PSUM evacuation alternates between
vector and scalar engines.
"""

import numpy as np
import ml_dtypes

import concourse.bass as bass
import concourse.mybir as mybir
import concourse.tile as tile
from concourse.tile import ScopedClock
from concourse.bass_utils import run_bass_kernel_spmd

P = 128
B = 8
TOKENS = 2048
D_IN = 4096
D_OUT = 4096
R = 256
I_CHUNKS = D_IN // P  # 32
R_HALVES = R // P  # 2
G = 4  # token groups (pipeline stages)
NT = TOKENS // G  # 512 tokens per group
TB = NT // P  # 4 token blocks per group
O_TILE = 512
O_TILES = D_OUT // O_TILE  # 8
O_WAVE = 3  # out-tiles per PSUM wave in stage 2 (6 psy bufs = 2 waves in flight)
X_SUBS = 4  # sub-DMAs per group's x tile (1 MiB each, contiguous in DRAM)
SUB_C = I_CHUNKS // X_SUBS  # i-chunks per sub-DMA
WARM_MM = 10  # N=512 zero matmuls at t~8us bridging until first x/vs data (~13us)
WARM_FINE = 16  # N=128 trailing warmups: fine quanta near expected data arrival;
# each costs only ~110ns of overshoot when DMA is early, but together they
# cover the slow-DMA runs where a >3.4us gap would re-throttle the clock gate
FILL_MM = 4  # zero matmuls after each DMA-paced chunk octet (keep HAM open)


def _patched_drain_and_barrier(self, tick_clock, wait_clock):
    # This walrus build's CoreV3 CTRL lowering accepts at most one sync-wait
    # on the TileContext-exit SP Drain; split the global-clock waits across a
    # chain of SP nops (one wait each) emitted just before the drain.
    nc = self.nc
    lead = nc.sync.nop(nofuse=True, hint="tile_drain_wait_split")
    wait_clock.add_sem_waits(lead.ins, ScopedClock({None: tick_clock.global_clock}))
    si = lead.ins.sync_info
    waits = list(si.on_wait or [])
    if len(waits) > 1:
        si.on_wait = waits[:1]
        for w in waits[1:]:
            extra = nc.sync.nop(nofuse=True, hint="tile_drain_wait_split")
            esi = extra.ins.sync_info
            if esi is None:
                extra.ins.sync_info = mybir.SyncInfo(on_wait=[w], on_update=[])
            else:
                esi.on_wait = [w]
    nc.sync.drain()
    nc.all_engine_barrier()
    assert self.sems is not None
    popped = nc._tile_sem_poison_stack.pop()
    assert popped is self._sem_poison
    nc.clear_and_free_semaphores(list(self.sems.allocated().values()))
    nc.all_engine_barrier()


def _install_drain_patch():
    if not getattr(tile.TileContext, "_drain_patch_installed", False):
        tile.TileContext._drain_and_barrier = _patched_drain_and_barrier
        tile.TileContext._drain_patch_installed = True


def _legalize_waits(nc):
    # This walrus build accepts at most one sync-wait per instruction.
    # Hoist extra waits onto same-engine nops inserted just before the
    # offending instruction (same engine queue -> identical blocking).
    for fn in nc.m.functions:
        for bb in fn.blocks:
            new_list = []
            for inst in list(bb.instructions):
                si = inst.sync_info
                waits = list(si.on_wait) if si and si.on_wait else []
                if len(waits) > 1:
                    for w in waits[:-1]:
                        nop = nc.engines[inst.engine].nop(
                            nofuse=True, hint="wait_split"
                        )
                        cur = nc.cur_bb.bb.instructions
                        assert cur[-1] is nop.ins
                        cur.pop()
                        nsi = nop.ins.sync_info
                        if nsi is None:
                            nop.ins.sync_info = mybir.SyncInfo(
                                on_wait=[w], on_update=[]
                            )
                        else:
                            nsi.on_wait = [w]
                        new_list.append(nop.ins)
                    si.on_wait = [waits[-1]]
                new_list.append(inst)
            bb.instructions[:] = new_list


def _build(iodt=mybir.dt.bfloat16):
    f32 = mybir.dt.float32
    nc = bass.Bass()
    # xg rows: (group, sub, partition); cols: i-chunk-within-sub then token.
    # Each sub-DMA then reads a fully contiguous DRAM block (line-rate HBM).
    xg_d = nc.declare_dram_parameter(
        "xg", [G * X_SUBS * P, SUB_C * NT], iodt, isOutput=False
    )
    vs_d = nc.declare_dram_parameter("vs", [R_HALVES * P, I_CHUNKS * P], iodt, isOutput=False)
    ut_d = nc.declare_dram_parameter("ut", [P, R_HALVES, D_OUT], iodt, isOutput=False)
    y_d = nc.declare_dram_parameter("y", [TOKENS, D_OUT], iodt, isOutput=True)

    with tile.TileContext(nc) as tc:
        with (
            tc.tile_pool(name="consts", bufs=1) as consts,
            tc.tile_pool(name="xp", bufs=2) as xp,
            tc.tile_pool(name="tp", bufs=2) as tp,
            tc.tile_pool(name="yp", bufs=6) as yp,
            tc.tile_pool(name="pst", bufs=2, space="PSUM") as pst,
            tc.tile_pool(name="psy", bufs=6, space="PSUM") as psy,
        ):
            # PE warmup: ~3.4us of zero matmuls so the HAM clock-gate opens
            # to 2.4 GHz while the first weight/x DMAs are still in flight
            zt = consts.tile([P, P + O_TILE], iodt)
            nc.gpsimd.memset(zt[:], 0.0)
            ps_warm = psy.tile([P, O_TILE], f32, tag="py", name="py")

            def fill_mm(n, ncols=O_TILE):
                for _ in range(n):
                    nc.tensor.matmul(
                        ps_warm[:, :ncols], zt[:, :P], zt[:, P : P + ncols],
                        start=True, stop=True,
                    )

            # long quanta build the >=3.4us busy streak the HAM needs; short
            # trailing quanta land near data-arrival so overshoot is cheap
            fill_mm(WARM_MM)
            fill_mm(WARM_FINE, ncols=P)

            vs_sb = consts.tile([P, R_HALVES, I_CHUNKS * P], iodt)
            ut_sb = consts.tile([P, R_HALVES, D_OUT], iodt)
            xg_sb = [None] * G
            t_sb = [None] * G
            w = SUB_C * NT

            def xsub(g, q, eng, pieces=None):
                row = (g * X_SUBS + q) * P
                if pieces:
                    # split at the given column fractions (eighths of the sub)
                    for a, b in pieces:
                        eng.dma_start(
                            out=xg_sb[g][:, q * w + a * w // 8 : q * w + b * w // 8],
                            in_=xg_d[row : row + P, a * w // 8 : b * w // 8],
                        )
                else:
                    eng.dma_start(
                        out=xg_sb[g][:, q * w : (q + 1) * w],
                        in_=xg_d[row : row + P, :],
                    )

            # Ramp: both rings feed stage-1 in exact consumption order.
            # SP:  x0s0a x0s0b x0s1 x0s3 | x1s0 x1s2 | xg2 xg3 | late stores
            # ACT: vs0a vs0b x0s2 vs1    | x1s1 x1s3 | ut | stores g0-1
            # The first x/vs pieces are tiny (128 KiB) so the first real
            # matmuls land ~2us earlier and chain straight off the warmup.
            xg_sb[0] = xp.tile([P, I_CHUNKS * NT], iodt, tag="xg", name="xg")
            xg_sb[1] = xp.tile([P, I_CHUNKS * NT], iodt, tag="xg", name="xg")
            vw = I_CHUNKS * P // 8
            xsub(0, 0, nc.sync, pieces=[(0, 2), (2, 4), (4, 8)])
            for a, b in [(0, 1), (1, 4), (4, 8)]:
                nc.scalar.dma_start(
                    out=vs_sb[:, 0, a * vw : b * vw],
                    in_=vs_d[0:P, a * vw : b * vw],
                )
            xsub(0, 1, nc.sync)
            xsub(0, 2, nc.scalar)
            xsub(0, 3, nc.sync)
            nc.scalar.dma_start(out=vs_sb[:, 1, :], in_=vs_d[P : 2 * P, :])
            xsub(1, 0, nc.sync)
            xsub(1, 1, nc.scalar)
            xsub(1, 2, nc.sync)
            xsub(1, 3, nc.scalar)
            nc.scalar.dma_start(out=ut_sb[:], in_=ut_d[:])

            def prefetch(g, eng):
                xg_sb[g] = xp.tile([P, I_CHUNKS * NT], iodt, tag="xg", name="xg")
                for q in range(X_SUBS):
                    xsub(g, q, eng)

            def stage1(g, fill_n=0):
                ps_t = [pst.tile([P, NT], f32, tag="pt", name="pt") for _ in range(R_HALVES)]
                xg = xg_sb[g]
                for h in range(R_HALVES):
                    for c in range(I_CHUNKS):
                        nc.tensor.matmul(
                            ps_t[h],
                            vs_sb[:, h, c * P : (c + 1) * P],
                            xg[:, c * NT : (c + 1) * NT],
                            start=(c == 0),
                            stop=(c == I_CHUNKS - 1),
                        )
                        # during the DMA-paced ramp, pad the x-sub-DMA wait at
                        # each octet boundary with zero matmuls so the PE never
                        # idles past the HAM MID window (would re-throttle)
                        if fill_n and h == 0 and c % SUB_C == SUB_C - 1 and c != I_CHUNKS - 1:
                            fill_mm(fill_n)
                t_sb[g] = tp.tile([P, R_HALVES, NT], iodt, tag="t", name="t")
                for h in range(R_HALVES):
                    nc.vector.tensor_copy(out=t_sb[g][:, h, :], in_=ps_t[h])

            def stage2(g):
                # late-pipeline stores dispatch from the SP ring (loads done by
                # then) so the ACT engine isn't saturated by copies + dispatches
                st_eng = nc.scalar if g < 2 else nc.sync
                for tb in range(TB):
                    y_sb = yp.tile([P, D_OUT], iodt, tag="yt", name="yt")
                    tsl = t_sb[g]
                    row = (g * TB + tb) * P
                    waves = [
                        list(range(i, min(i + O_WAVE, O_TILES)))
                        for i in range(0, O_TILES, O_WAVE)
                    ]
                    for wi, ots in enumerate(waves):
                        ps_y = {}
                        for h in range(R_HALVES):
                            for ot in ots:
                                if h == 0:
                                    # JIT alloc: each slot claimed right before
                                    # its first matmul so the wave can start as
                                    # prior-wave copies retire one by one
                                    ps_y[ot] = psy.tile(
                                        [P, O_TILE], f32, tag="py", name="py"
                                    )
                                nc.tensor.matmul(
                                    ps_y[ot],
                                    tsl[:, h, tb * P : (tb + 1) * P],
                                    ut_sb[:, h, ot * O_TILE : (ot + 1) * O_TILE],
                                    start=(h == 0),
                                    stop=(h == R_HALVES - 1),
                                )
                        for ot in ots:
                            dst = y_sb[:, ot * O_TILE : (ot + 1) * O_TILE]
                            # both engines evacuate one half each: the PSUM
                            # slot releases ~2x sooner, so the next wave's
                            # matmuls aren't gated on a single 600ns copy
                            hw = O_TILE // 2
                            if ot % 2 == 0:
                                nc.vector.tensor_copy(out=dst[:, :hw], in_=ps_y[ot][:, :hw])
                                nc.scalar.copy(out=dst[:, hw:], in_=ps_y[ot][:, hw:])
                            else:
                                nc.scalar.copy(out=dst[:, :hw], in_=ps_y[ot][:, :hw])
                                nc.vector.tensor_copy(out=dst[:, hw:], in_=ps_y[ot][:, hw:])
                        if g == G - 1 and tb == TB - 1:
                            # very last block: store each wave's slice as soon
                            # as its copies land, on alternating rings, so the
                            # kernel tail overlaps the final compute
                            c0, c1 = ots[0] * O_TILE, (ots[-1] + 1) * O_TILE
                            weng = nc.sync if wi % 2 == 0 else nc.scalar
                            weng.dma_start(
                                out=y_d[row : row + P, c0:c1],
                                in_=y_sb[:, c0:c1],
                            )
                    if not (g == G - 1 and tb == TB - 1):
                        st_eng.dma_start(out=y_d[row : row + P, :], in_=y_sb[:])

            # software pipeline: s1(0) s1(1) s2(0) s1(2) s2(1) s1(3) s2(2) s2(3)
            # xg(1) rides the ACT ring (idle after weights until stores begin)
            # so both rings pull x during the ramp
            stage1(0, fill_n=FILL_MM)
            stage1(1, fill_n=2)
            prefetch(2, nc.sync)
            stage2(0)
            stage1(2)
            prefetch(3, nc.sync)
            stage2(1)
            stage1(3)
            stage2(2)
            stage2(3)

    _legalize_waits(nc)
    return nc


_CACHED = {}


def kernel(x, u_approx, s_approx, v_approx, _trace=False):
    _install_drain_patch()
    bf16 = ml_dtypes.bfloat16

    vp = v_approx.astype(np.float32) * s_approx.astype(np.float32)[None, :]
    # vs[h*128+p, c*128+rr] = vp[c*128+p, h*128+rr]
    vs_host = np.ascontiguousarray(
        vp.reshape(I_CHUNKS, P, R_HALVES, P).transpose(2, 1, 0, 3).reshape(
            R_HALVES * P, I_CHUNKS * P
        )
    ).astype(bf16)
    # ut[p, h, o] = u[o, h*128+p]
    ut_host = np.ascontiguousarray(
        np.ascontiguousarray(u_approx.T).reshape(R_HALVES, P, D_OUT).transpose(1, 0, 2)
    ).astype(bf16)
    # xg[((g*X_SUBS+q)*128)+p, cc*512+n] = x[b, g*512+n, (q*SUB_C+cc)*128+p]
    xg = [
        np.ascontiguousarray(
            x[b]
            .reshape(G, NT, X_SUBS, SUB_C, P)
            .transpose(0, 2, 4, 3, 1)
            .reshape(G * X_SUBS * P, SUB_C * NT)
        ).astype(bf16)
        for b in range(B)
    ]
    in_maps = [{"xg": xg[b], "vs": vs_host, "ut": ut_host} for b in range(B)]

    if "nc" not in _CACHED:
        _CACHED["nc"] = _build()
    res = run_bass_kernel_spmd(_CACHED["nc"], in_maps, list(range(B)), trace=_trace)
    y = np.stack([res.results[b]["y"].astype(np.float32) for b in range(B)])
    if _trace:
        kernel.last_exec_time_ns = res.exec_time_ns
    return y
